# revision 1
# baseline (speedup 1.0000x reference)
"""ChildSumTreeLSTM (8191-node complete binary tree) on 8 Trainium2 cores.

Strategy
--------
Scan order t = N-1-j over level-order heap indices j makes the recurrence
level-parallel: every node of tree-level L depends only on level L+1. The
tree splits into 8 independent subtrees rooted at the 8 level-3 nodes; each
NeuronCore computes one subtree's levels 12..8 (992 nodes) entirely in
SBUF. The host computes the tiny top levels 7..0 (255 nodes) in fp32 numpy.

Device layout is transposed: [feature-on-partition, node-on-free], so the
shrinking levels keep the full 128-wide PE array busy (M = feature chunks
of 128, N = nodes). Matmul operands are fp16 (1 PE cycle/row like bf16 —
4x for fp32 — but with a 10-bit mantissa, ~8x less rounding error);
accumulation and all state are fp32 in PSUM/SBUF.

Each level is split into STREAMS=2 blocks (independent sub-subtrees) whose
serial ACT/DVE eviction chains hide behind each other's matmuls, and
within a block children are planar ([left kids | right kids] in the next
level), keeping every pairwise op contiguous.

Per level and node-group (G<=128 columns):
  phase 1 (dependency-free, overlaps the chunked input DMAs):
    iou gate psums [128,4,G] += WiouxT(kc) @ XT(cols);  xf psum likewise
  phase 2:  hsum = hL + hR;  iou psums += WiouhT(q) @ hsum
    fL/fR psums = WfhT(q) @ h_child;  gates = ACT(sigmoid/tanh) evictions
    f = sigmoid(f_psum + xf);  c = i*u + fL*cL + fR*cR;  h = o*tanh(c)
"""

import os
import numpy as np

N_NODES = 8191
IN_DIM = 1024
MEM_DIM = 512
N_CORES = 8
MIN_DEV_LEVEL = int(os.environ.get("KERNEL_MIN_DEV_LEVEL", "8"))
DEV_LEVELS = list(range(12, MIN_DEV_LEVEL - 1, -1))
LVL_N = [1 << (l - 3) for l in DEV_LEVELS]          # per-subtree level sizes
LVL_OFF = np.concatenate([[0], np.cumsum(LVL_N)])[:len(LVL_N)].tolist()
NCOLS = int(sum(LVL_N))
G = 128                                 # node-group width
KC_X = IN_DIM // 128                    # 8 K-chunks over in_dim
KC_H = MEM_DIM // 128                   # 4 K-chunks over mem_dim
NM_IOU = 3 * MEM_DIM // 128             # 12 M-chunks over 3*mem_dim
NM_F = MEM_DIM // 128                   # 4 M-chunks over mem_dim


STREAMS = int(os.environ.get("KERNEL_STREAMS", "2"))


def _level_orders():
    """Per-level column orders: each level is split into STREAMS blocks
    (independent sub-subtrees, so one stream's serial eviction chain hides
    behind another stream's matmuls), and within a block children are
    planar: children of the node at block position a sit at positions a
    (left) and ns (block size) + a (right) of the next level's block.
    ord[l][j] = within-level natural node index in column j."""
    orders = {}
    top = DEV_LEVELS[-1]
    orders[top] = np.arange(1 << (top - 3))
    for l in range(top + 1, 13):
        p = orders[l - 1]
        ns = len(p) // STREAMS
        blocks = []
        for s in range(STREAMS):
            blk = p[s * ns:(s + 1) * ns]
            blocks.append(np.concatenate([2 * blk, 2 * blk + 1]))
        orders[l] = np.concatenate(blocks)
    return orders


_ORDERS = _level_orders()
# device column -> (level natural node index) permutation, per level-major col
COLPERM = np.concatenate(
    [LVL_OFF[li] + _ORDERS[l] for li, l in enumerate(DEV_LEVELS)])


def _sigmoid(x):
    out = np.empty_like(x)
    np.negative(x, out=out)
    np.exp(out, out=out)
    out += 1.0
    np.reciprocal(out, out=out)
    return out


def _expected_tree():
    j = np.arange(N_NODES)
    c1, c2 = 2 * j + 1, 2 * j + 2
    m1, m2 = c1 < N_NODES, c2 < N_NODES
    idx = np.stack([np.where(m1, N_NODES - 1 - c1, 0),
                    np.where(m2, N_NODES - 1 - c2, 0)], axis=1)[::-1]
    msk = np.stack([m1, m2], axis=1).astype(np.float32)[::-1]
    return np.ascontiguousarray(idx.astype(np.int32)), np.ascontiguousarray(msk)


def _subtree_t_cols(core):
    """Scan indices t of core's subtree nodes in device column order
    (level-major, leaves first, planar-children order within levels)."""
    j0 = 7 + core
    js = []
    for l in DEV_LEVELS:
        d = l - 3
        base = (j0 + 1) * (1 << d) - 1
        js.append(base + _ORDERS[l])
    return N_NODES - 1 - np.concatenate(js)


# ---------------------------------------------------------------------------
# Bass program (built once, cached)
# ---------------------------------------------------------------------------
_NC_CACHE = {}


def _build_nc(b_iou_const, b_fx_const):
    import concourse.bass as bass
    import concourse.mybir as mybir
    import concourse.tile as tile
    key = (b_iou_const, b_fx_const)
    if key in _NC_CACHE:
        return _NC_CACHE[key]

    f32 = mybir.dt.float32
    f16 = mybir.dt.float16
    ACT = mybir.ActivationFunctionType

    nc = bass.Bass("TRN2", target_bir_lowering=False, debug=False)
    xt_d = nc.declare_dram_parameter("xt", [128, KC_X, NCOLS], f16, isOutput=False)
    wioux_d = nc.declare_dram_parameter("wioux", [128, KC_X, 3 * MEM_DIM], f16, isOutput=False)
    wiouh_d = nc.declare_dram_parameter("wiouh", [128, KC_H, 3 * MEM_DIM], f16, isOutput=False)
    wfx_d = nc.declare_dram_parameter("wfx", [128, KC_X, MEM_DIM], f16, isOutput=False)
    wfh_d = nc.declare_dram_parameter("wfh", [128, KC_H, MEM_DIM], f16, isOutput=False)
    c_out_d = nc.declare_dram_parameter("c_out", [128, KC_H, NCOLS], f32, isOutput=True)
    h_out_d = nc.declare_dram_parameter("h_out", [128, KC_H, NCOLS], f32, isOutput=True)

    with tile.TileContext(nc) as tc:
        from contextlib import ExitStack
        with ExitStack() as ctx:
            const = ctx.enter_context(tc.tile_pool(name="const", bufs=1))
            state = ctx.enter_context(tc.tile_pool(name="state", bufs=1))
            work = ctx.enter_context(tc.tile_pool(name="work", bufs=2))
            ps_iou = ctx.enter_context(tc.tile_pool(name="ps_iou", bufs=2, space="PSUM"))
            ps_f = ctx.enter_context(tc.tile_pool(name="ps_f", bufs=1, space="PSUM"))
            ps_xf = ctx.enter_context(tc.tile_pool(name="ps_xf", bufs=1, space="PSUM"))

            wioux = const.tile([128, KC_X, 3 * MEM_DIM], f16)
            xt = const.tile([128, KC_X, NCOLS], f16)
            wfx = const.tile([128, KC_X, MEM_DIM], f16)
            wiouh = const.tile([128, KC_H, 3 * MEM_DIM], f16)
            wfh = const.tile([128, KC_H, MEM_DIM], f16)
            # Split the two big startup loads per K-chunk so PE can begin
            # the first leaf accumulation a couple us in instead of ~18us.
            # Only the leaf columns of X are on the startup critical path.
            n_leaf = LVL_N[0]
            for kc in range(0, KC_X):
                nc.sync.dma_start(out=wioux[:, kc, :], in_=wioux_d[:, kc, :])
                nc.sync.dma_start(out=xt[:, kc, 0:n_leaf],
                                  in_=xt_d[:, kc, 0:n_leaf])
            nc.sync.dma_start(out=xt[:, 0:4, n_leaf:],
                              in_=xt_d[:, 0:4, n_leaf:])
            nc.sync.dma_start(out=xt[:, 4:8, n_leaf:],
                              in_=xt_d[:, 4:8, n_leaf:])
            nc.sync.dma_start(out=wfx[:], in_=wfx_d[:])
            nc.sync.dma_start(out=wiouh[:], in_=wiouh_d[:])
            nc.sync.dma_start(out=wfh[:], in_=wfh_d[:])

            hbf = state.tile([128, KC_H, NCOLS], f16)
            cT = state.tile([128, KC_H, NCOLS], f32)
            hT = state.tile([128, KC_H, NCOLS], f32)

            gate_fn = [ACT.Sigmoid, ACT.Sigmoid, ACT.Tanh]
            pending_casts = []

            for li, l in enumerate(DEV_LEVELS):
                n = LVL_N[li]
                off = LVL_OFF[li]
                leaf = (l == 12)
                ch_off = None if leaf else LVL_OFF[li - 1]
                # groups: (col0, gn, chL, chR) per stream chunk, interleaved
                # across streams so one stream's eviction chain overlaps the
                # next stream's matmuls
                ns = n // STREAMS
                groups = []
                for a in range(0, ns, G):
                    for s in range(STREAMS):
                        gn = min(G, ns - a)
                        col0 = off + s * ns + a
                        if leaf:
                            chL = chR = None
                        else:
                            chL = ch_off + s * 2 * ns + a
                            chR = chL + ns
                        groups.append((col0, gn, chL, chR))

                ps_tiles = []
                # Phase 1: x-side matmuls (iou partial sums and, for
                # internal levels, xf) — all dependency-free. Emitted in
                # waves of 2 groups with kc outermost across the wave, so
                # the per-K-chunk input DMAs pipeline against enough PE
                # work (PSUM pools hold 2 groups per gate tag).
                for w0 in range(0, len(groups), 2):
                    wave = groups[w0:w0 + 2]
                    wave_ps = []
                    for (g0, gn, _, _) in wave:
                        ps = [ps_iou.tile([128, 4, gn], f32, tag=f"ps_{g}",
                                          name=f"ps_{g}",
                                          bufs=(1 if g == "u" else 2))
                              for g in "iou"]
                        psxf = None
                        if not leaf:
                            psxf = ps_xf.tile([128, NM_F, gn], f32,
                                              tag="ps_xf", name="ps_xf")
                        wave_ps.append((ps, psxf))
                        ps_tiles.append((ps, psxf))
                    for kc in range(KC_X):
                        for wi, (g0, gn, _, _) in enumerate(wave):
                            ps, psxf = wave_ps[wi]
                            for gt in range(3):
                                for mi in range(4):
                                    m = 4 * gt + mi
                                    nc.tensor.matmul(
                                        ps[gt][:, mi, :],
                                        lhsT=wioux[:, kc,
                                                   m * 128:(m + 1) * 128],
                                        rhs=xt[:, kc, g0:g0 + gn],
                                        start=(kc == 0 and mi == 0),
                                        stop=(leaf and kc == KC_X - 1
                                              and mi == 3),
                                    )
                            if psxf is not None:
                                for m in range(NM_F):
                                    nc.tensor.matmul(
                                        psxf[:, m, :],
                                        lhsT=wfx[:, kc,
                                                 m * 128:(m + 1) * 128],
                                        rhs=xt[:, kc, g0:g0 + gn],
                                        start=(kc == 0 and m == 0),
                                        stop=(kc == KC_X - 1
                                              and m == NM_F - 1),
                                    )

                # Phase 2: recurrence + evictions per group
                for gi, (g0, gn, chL, chR) in enumerate(groups):
                    ps, psxf = ps_tiles[gi]
                    if not leaf:
                        # child h sum straight from fp32 hT (fp16 out),
                        # contiguous thanks to the planar child layout
                        hsum = work.tile([128, KC_H, gn], f16, tag="hsum")
                        nc.vector.tensor_add(
                            hsum[:],
                            hT[:, :, chL:chL + gn],
                            hT[:, :, chR:chR + gn],
                        )
                        # fp16 copies of the previous level's h (f-gate rhs;
                        # off the critical chain, emitted after hsum)
                        for (pc0, pcn) in pending_casts:
                            nc.scalar.copy(hbf[:, :, pc0:pc0 + pcn],
                                           hT[:, :, pc0:pc0 + pcn])
                        pending_casts = []
                        for gt in range(3):
                            for mi in range(4):
                                m = 4 * gt + mi
                                for q in range(KC_H):
                                    nc.tensor.matmul(
                                        ps[gt][:, mi, :],
                                        lhsT=wiouh[:, q, m * 128:(m + 1) * 128],
                                        rhs=hsum[:, q, :],
                                        start=False,
                                        stop=(mi == 3 and q == KC_H - 1),
                                    )
                        # f gate, one psum bank per child side: W_fh @ h_child
                        # (the W_fx @ X part comes from psxf at eviction)
                        psfL = ps_f.tile([128, NM_F, gn], f32, tag="ps_fL",
                                         name="ps_fL")
                        psfR = ps_f.tile([128, NM_F, gn], f32, tag="ps_fR",
                                         name="ps_fR")
                        for psf_h, ch in ((psfL, chL), (psfR, chR)):
                            for m in range(NM_F):
                                for q in range(KC_H):
                                    nc.tensor.matmul(
                                        psf_h[:, m, :],
                                        lhsT=wfh[:, q, m * 128:(m + 1) * 128],
                                        rhs=hbf[:, q, ch:ch + gn],
                                        start=(m == 0 and q == 0),
                                        stop=(m == NM_F - 1
                                              and q == KC_H - 1),
                                    )

                    # evictions, ordered for the critical chain:
                    # i, u first (feed iu), then f, then o, then tanh(c)
                    i_sb = work.tile([128, 4, gn], f32, tag="sb_i")
                    u_sb = work.tile([128, 4, gn], f32, tag="sb_u")
                    o_sb = work.tile([128, 4, gn], f32, tag="sb_o")
                    nc.scalar.activation(i_sb[:], ps[0][:], gate_fn[0],
                                         bias=b_iou_const[0])
                    nc.scalar.activation(u_sb[:], ps[2][:], gate_fn[2],
                                         bias=b_iou_const[2])
                    nc.scalar.activation(o_sb[:], ps[1][:], gate_fn[1],
                                         bias=b_iou_const[1])

                    cw = cT[:, :, g0:g0 + gn]
                    if leaf:
                        nc.vector.tensor_mul(cw, i_sb[:], u_sb[:])
                    else:
                        iu = work.tile([128, 4, gn], f32, tag="iu")
                        nc.vector.tensor_mul(iu[:], i_sb[:], u_sb[:])
                        xf_sb = work.tile([128, NM_F, gn], f32, tag="xf_sb")
                        nc.scalar.copy(xf_sb[:], psxf[:])
                        fzL = work.tile([128, NM_F, gn], f32, tag="fzL")
                        nc.vector.tensor_add(fzL[:], psfL[:], xf_sb[:])
                        fzR = work.tile([128, NM_F, gn], f32, tag="fzR")
                        nc.vector.tensor_add(fzR[:], psfR[:], xf_sb[:])
                        f_sbL = work.tile([128, NM_F, gn], f32, tag="f_sbL")
                        nc.scalar.activation(f_sbL[:], fzL[:], ACT.Sigmoid,
                                             bias=b_fx_const)
                        f_sbR = work.tile([128, NM_F, gn], f32, tag="f_sbR")
                        nc.scalar.activation(f_sbR[:], fzR[:], ACT.Sigmoid,
                                             bias=b_fx_const)
                        fcL = work.tile([128, NM_F, gn], f32, tag="fcL")
                        nc.vector.tensor_mul(fcL[:], f_sbL[:],
                                             cT[:, :, chL:chL + gn])
                        fcR = work.tile([128, NM_F, gn], f32, tag="fcR")
                        nc.vector.tensor_mul(fcR[:], f_sbR[:],
                                             cT[:, :, chR:chR + gn])
                        fcs = work.tile([128, NM_F, gn], f32, tag="fcs")
                        nc.vector.tensor_add(fcs[:], fcL[:], fcR[:])
                        nc.vector.tensor_add(cw, iu[:], fcs[:])
                    if l == DEV_LEVELS[-1]:
                        # top device level: h = o*tanh(c) finishes on the
                        # host; ship o in the h slot straight from the
                        # eviction tile, on the otherwise-idle ACT ring
                        nc.sync.dma_start(out=h_out_d[:, :, g0:g0 + gn],
                                           in_=o_sb[:])
                    else:
                        tanh_c = work.tile([128, 4, gn], f32, tag="tanh_c")
                        nc.scalar.activation(tanh_c[:], cw, ACT.Tanh,
                                             bias=0.0)
                        nc.vector.tensor_mul(hT[:, :, g0:g0 + gn], o_sb[:],
                                             tanh_c[:])
                    if l != DEV_LEVELS[-1]:
                        pending_casts.append((g0, gn))

                nc.sync.dma_start(out=c_out_d[:, :, off:off + n],
                                  in_=cT[:, :, off:off + n])
                if l != DEV_LEVELS[-1]:
                    nc.sync.dma_start(out=h_out_d[:, :, off:off + n],
                                      in_=hT[:, :, off:off + n])

    _split_multiwait(nc)
    _NC_CACHE[key] = nc
    return nc


def _split_multiwait(nc, limit=1):
    """This walrus build rejects instructions with more than `limit` sync
    waits. Engines execute their queues in order, so excess waits can move
    onto freshly inserted same-engine NOPs placed immediately before the
    offending instruction."""
    import concourse.mybir as mybir
    f = nc.m.functions[0]
    for bb in list(f.blocks):
        insts = list(bb.instructions)
        out = []
        changed = False
        for inst in insts:
            si = inst.sync_info
            waits = list(si.on_wait) if si and si.on_wait else []
            if len(waits) > limit:
                changed = True
                for w in waits[:-limit]:
                    bi = nc.engines[inst.engine].nop(nofuse=True)
                    nop_ins = bi.ins
                    ab = nc.cur_bb.bb
                    assert ab.instructions[-1].name == nop_ins.name
                    ab.instructions = ab.instructions[:-1]
                    nop_ins.sync_info = mybir.SyncInfo(on_wait=[w],
                                                       on_update=[])
                    out.append(nop_ins)
                inst.sync_info = mybir.SyncInfo(
                    on_wait=waits[-limit:], on_update=list(si.on_update or []))
            out.append(inst)
        if changed:
            bb.instructions = out


# ---------------------------------------------------------------------------
# Host side
# ---------------------------------------------------------------------------
def _prep_in_maps(inputs, W_ioux, W_iouh, W_fx, W_fh):
    bf = np.float16

    def kmaj(w):  # [out, in] -> [128, in//128, out] fp16
        return np.ascontiguousarray(
            w.T.reshape(w.shape[1] // 128, 128, w.shape[0]).transpose(1, 0, 2)
        ).astype(bf)

    shared = {
        "wioux": kmaj(W_ioux), "wiouh": kmaj(W_iouh),
        "wfx": kmaj(W_fx), "wfh": kmaj(W_fh),
    }
    maps = []
    for core in range(N_CORES):
        tcols = _subtree_t_cols(core)
        X = inputs[tcols]                                 # [NCOLS, IN_DIM]
        xt = np.ascontiguousarray(
            X.reshape(NCOLS, KC_X, 128).transpose(2, 1, 0)).astype(bf)
        maps.append({"xt": xt, **shared})
    return maps


def _run_device(inputs, W_ioux, b_ioux, W_iouh, W_fx, b_fx, W_fh,
                trace=False):
    from concourse.bass_utils import run_bass_kernel_spmd
    b_iou_const = tuple(
        float(b_ioux[k * MEM_DIM]) for k in range(3))
    nc = _build_nc(b_iou_const, float(b_fx[0]))
    in_maps = _prep_in_maps(inputs, W_ioux, W_iouh, W_fx, W_fh)
    res = run_bass_kernel_spmd(nc, in_maps, list(range(N_CORES)),
                               trace=trace)
    c_all = np.zeros((N_NODES, MEM_DIM), np.float32)
    h_all = np.zeros((N_NODES, MEM_DIM), np.float32)
    n_top = LVL_N[-1]
    for core in range(N_CORES):
        tcols = _subtree_t_cols(core)
        co = res.results[core]["c_out"]       # [128, 4, NCOLS]
        ho = res.results[core]["h_out"]       # h, except o for the top level
        c = co.transpose(2, 1, 0).reshape(NCOLS, MEM_DIM)
        h = ho.transpose(2, 1, 0).reshape(NCOLS, MEM_DIM).copy()
        h[-n_top:] = h[-n_top:] * np.tanh(c[-n_top:])
        c_all[tcols] = c
        h_all[tcols] = h
    return c_all, h_all, res


def _host_top(c_all, h_all, inputs, W_ioux, b_ioux, W_iouh, W_fx, b_fx, W_fh):
    for l in range(MIN_DEV_LEVEL - 1, -1, -1):
        js = np.arange((1 << l) - 1, (1 << (l + 1)) - 1)
        ts = N_NODES - 1 - js
        t1 = N_NODES - 1 - (2 * js + 1)
        t2 = N_NODES - 1 - (2 * js + 2)
        X = inputs[ts]
        xiou = X @ W_ioux.T + b_ioux
        xf = X @ W_fx.T + b_fx
        h1, h2 = h_all[t1], h_all[t2]
        iou = xiou + (h1 + h2) @ W_iouh.T
        i, o, u = np.split(iou, 3, axis=1)
        i, o, u = _sigmoid(i), _sigmoid(o), np.tanh(u)
        f1 = _sigmoid(h1 @ W_fh.T + xf)
        f2 = _sigmoid(h2 @ W_fh.T + xf)
        c = i * u + f1 * c_all[t1] + f2 * c_all[t2]
        h = o * np.tanh(c)
        c_all[ts], h_all[ts] = c, h
    return c_all, h_all


def _numpy_fallback(inputs, W_ioux, b_ioux, W_iouh, W_fx, b_fx, W_fh,
                    children_idx, children_mask):
    """Exact generic path for arbitrary children_idx/mask: level-batched
    over the effective DAG (a child contributes nothing when masked or when
    its index >= t, since state reads-before-write are zero)."""
    N = inputs.shape[0]
    t = np.arange(N)[:, None]
    eff = (children_mask != 0) & (children_idx < t)
    depth = np.zeros(N, np.int64)
    for tt in range(N):
        ds = depth[children_idx[tt]][eff[tt]]
        depth[tt] = 0 if ds.size == 0 else ds.max() + 1
    xiou = inputs @ W_ioux.T + b_ioux
    xf = inputs @ W_fx.T + b_fx
    c_all = np.zeros((N, MEM_DIM), np.float32)
    h_all = np.zeros((N, MEM_DIM), np.float32)
    for d in range(depth.max() + 1):
        ts = np.where(depth == d)[0]
        ci = children_idx[ts]                  # [n, 2]
        m = (eff[ts]).astype(np.float32)[:, :, None]
        hch = h_all[ci] * m                    # [n, 2, M]
        cch = c_all[ci] * m
        iou = xiou[ts] + hch.sum(1) @ W_iouh.T
        i, o, u = np.split(iou, 3, axis=1)
        i, o, u = _sigmoid(i), _sigmoid(o), np.tanh(u)
        f = _sigmoid(np.einsum("ncm,km->nck", hch, W_fh) + xf[ts][:, None, :])
        c = i * u + (f * cch).sum(1)
        h = o * np.tanh(c)
        c_all[ts], h_all[ts] = c, h
    return c_all, h_all


def kernel(inputs, W_ioux, b_ioux, W_iouh, W_fx, b_fx, W_fh,
           children_idx, children_mask, _trace=False, _return_res=False):
    inputs = np.asarray(inputs, np.float32)
    W_ioux = np.asarray(W_ioux, np.float32)
    b_ioux = np.asarray(b_ioux, np.float32)
    W_iouh = np.asarray(W_iouh, np.float32)
    W_fx = np.asarray(W_fx, np.float32)
    b_fx = np.asarray(b_fx, np.float32)
    W_fh = np.asarray(W_fh, np.float32)
    children_idx = np.asarray(children_idx)
    children_mask = np.asarray(children_mask, np.float32)

    exp_idx, exp_msk = _expected_tree()
    biases_const = (np.all(b_ioux[:MEM_DIM] == b_ioux[0])
                    and np.all(b_ioux[MEM_DIM:2 * MEM_DIM] == b_ioux[MEM_DIM])
                    and np.all(b_ioux[2 * MEM_DIM:] == b_ioux[2 * MEM_DIM])
                    and np.all(b_fx == b_fx[0]))
    if (not np.array_equal(children_idx, exp_idx)
            or not np.array_equal(children_mask, exp_msk)
            or not biases_const
            or os.environ.get("KERNEL_FORCE_FALLBACK")):
        c_all, h_all = _numpy_fallback(inputs, W_ioux, b_ioux, W_iouh,
                                       W_fx, b_fx, W_fh,
                                       children_idx, children_mask)
        return c_all, h_all

    c_all, h_all, res = _run_device(inputs, W_ioux, b_ioux, W_iouh,
                                    W_fx, b_fx, W_fh, trace=_trace)
    c_all, h_all = _host_top(c_all, h_all, inputs, W_ioux, b_ioux, W_iouh,
                             W_fx, b_fx, W_fh)
    if _return_res:
        return (c_all, h_all), res
    return c_all, h_all

